# revision 20
# baseline (speedup 1.0000x reference)
"""Causal multi-head self-attention (B=32, T=512, C=1024, H=16) on 8 trn2 cores.

Data-parallel over batch (4 items/core), identical NEFF on all cores.
All activations stay in [channel, token] layout so every matmul has its
contraction dim on partitions with no transposes:

  qT/kT = Wq/Wk-tiles.T @ xT-tiles          (bf16, 512-wide free dim)
  S_T   = K_slice.T @ Q_slice  [k, q]       (bf16; causal => shrink N per kt;
                                             head pairs in PE row groups)
  att   = exp(scale*S_T + pad_bias[k])      (ACT; pad mask as per-part. bias)
  att  *= causal_binmask (diag blocks)      (one merged DVE multiply per pair)
  y/den = [V | 1].T @ att                   (bf16; ones column -> denom)
  rec   = 1/den                             (DVE reciprocal_approx_fast)
  rb    = bcast(rec)                        (GpSimd partition_broadcast)
  yT    = av_psum * rb                      (DVE, fused evac+normalize)
  outT  = Wp-tiles.T @ yT + bp_eff          (bf16; bv folded into bp_eff)

Schedule: the Q/K projections of batch b+1 and the output projection of
batch b-1 are interleaved into the attention head-pair loop of batch b,
one m-group of each per head pair.  This balances the phases: the
attention loop is PE-bound with ACT/DVE slack, instead of a PE-dense
QKV phase followed by an ACT/DVE-bound attention phase.  The AV fill
runs one head pair behind the S/exp stage and the softmax-normalize
chain two behind, so no engine queue head-of-line blocks on a
just-emitted matmul.  gpsimd runs ONLY partition_broadcast — mixing in
any default-library op makes it thrash ucode libraries (LOAD_LIB /
UNLOAD_LIB with full drains every iteration).
"""

import sys

sys.path.insert(0, "/opt/trn_rl_repo")

import ml_dtypes
import numpy as np

import concourse.bass as bass
import concourse.tile as tile
from concourse import bacc, mybir

B, T, C, H = 32, 512, 1024, 16
D = C // H  # 64
N_CORES = 8
BL = B // N_CORES  # batches per core
NEG = -1.0e9

F32 = mybir.dt.float32
BF16 = mybir.dt.bfloat16
BF16_NP = ml_dtypes.bfloat16
AF = mybir.ActivationFunctionType
OP = mybir.AluOpType


def build_nc(c=C, t=T, bl=BL, h=H):
    """Build the per-core Bass program. Same NEFF runs on every core."""
    nct = c // 128   # channel tiles
    ktt = t // 128   # key/token tiles per sequence
    nch = (c + 511) // 512  # 512-wide output chunks for V projection
    scale = 1.0 / float(np.sqrt(D))

    nc = bacc.Bacc(None, target_bir_lowering=False)

    xTb = nc.dram_tensor("xTb", [c, bl * t], BF16, kind="ExternalInput")
    wq_t = nc.dram_tensor("wq_t", [c, c], BF16, kind="ExternalInput")
    wk_t = nc.dram_tensor("wk_t", [c, c], BF16, kind="ExternalInput")
    wv_t = nc.dram_tensor("wv_t", [c, c], BF16, kind="ExternalInput")
    wp_t = nc.dram_tensor("wp_t", [c, c], BF16, kind="ExternalInput")
    bq_t = nc.dram_tensor("bq_t", [128, nct], F32, kind="ExternalInput")
    bk_t = nc.dram_tensor("bk_t", [128, nct], F32, kind="ExternalInput")
    bpe_t = nc.dram_tensor("bpe_t", [128, nct], F32, kind="ExternalInput")
    pad_t = nc.dram_tensor("pad_t", [128, bl * ktt], F32, kind="ExternalInput")
    cmask = nc.dram_tensor("cmask", [128, ktt, 2, 128], BF16, kind="ExternalInput")
    outT = nc.dram_tensor("outT", [bl, c, t], BF16, kind="ExternalOutput")

    with tile.TileContext(nc) as tc:
        with (
            tc.tile_pool(name="weights", bufs=1) as wpool,
            tc.tile_pool(name="consts", bufs=1) as cpool,
            tc.tile_pool(name="acts", bufs=1) as apool,
            tc.tile_pool(name="att", bufs=2) as attp,
            tc.tile_pool(name="norm", bufs=2) as npool,
            tc.tile_pool(name="oevac", bufs=2) as opool,
            tc.tile_pool(name="psum", bufs=2, space=bass.MemorySpace.PSUM) as pp,
        ):
            # ---- consts (tiny) + wq + wk on sync; x(b0) + wv/wp on scalar —
            #      the two HWDGE queues load in parallel
            bq_sb = cpool.tile([128, nct], F32, tag="bq")
            bk_sb = cpool.tile([128, nct], F32, tag="bk")
            bpe_sb = cpool.tile([128, nct], F32, tag="bpe")
            pad_sb = cpool.tile([128, bl * ktt], F32, tag="pad")
            cm_sb = cpool.tile([128, ktt, 2, 128], BF16, tag="cmask")
            nc.sync.dma_start(bq_sb, bq_t[:])
            nc.sync.dma_start(bk_sb, bk_t[:])
            nc.sync.dma_start(bpe_sb, bpe_t[:])
            nc.sync.dma_start(pad_sb, pad_t[:])
            nc.sync.dma_start(cm_sb, cmask[:])

            def load_w(dram, tag, eng):
                tiles = [
                    wpool.tile([128, c], BF16, tag=f"{tag}{k}", name=f"{tag}{k}")
                    for k in range(nct)
                ]
                w_r = dram[:].rearrange("(k p) m -> p k m", p=128)
                for k in range(nct):
                    eng.dma_start(tiles[k], w_r[:, k, :])
                return tiles

            def load_x(b):
                tiles = [
                    apool.tile([128, t], BF16, tag=f"x{k}", bufs=2, name=f"x{b}_{k}")
                    for k in range(nct)
                ]
                x_r = xTb[:, b * t : (b + 1) * t].rearrange(
                    "(k p) n -> p k n", p=128
                )
                for k in range(nct):
                    nc.scalar.dma_start(tiles[k], x_r[:, k, :])
                return tiles

            def load_w_split(dram, tag):
                """Alternate k-tiles across both HWDGE queues."""
                tiles = [
                    wpool.tile([128, c], BF16, tag=f"{tag}{k}", name=f"{tag}{k}")
                    for k in range(nct)
                ]
                w_r = dram[:].rearrange("(k p) m -> p k m", p=128)
                for k in range(nct):
                    eng = nc.sync if k % 2 == 0 else nc.scalar
                    eng.dma_start(tiles[k], w_r[:, k, :])
                return tiles

            wq_k = load_w_split(wq_t, "wq")
            x_cur = load_x(0)
            wk_k = load_w_split(wk_t, "wk")
            wv_k = load_w(wv_t, "wv", nc.scalar)
            wp_k = load_w(wp_t, "wp", nc.sync)

            def alloc_qkT(b):
                qTn = [
                    apool.tile([128, t], BF16, tag=f"qT{m}", bufs=2,
                               name=f"qT{b}_{m}")
                    for m in range(nct)
                ]
                kTn = [
                    apool.tile([128, t], BF16, tag=f"kT{m}", bufs=2,
                               name=f"kT{b}_{m}")
                    for m in range(nct)
                ]
                return qTn, kTn

            def emit_lin_group(dst, w_tiles, x_tiles, m, b_sb):
                """dst[128,t] = (W.T @ x)[m-block] + bias, evac on ACT."""
                ps = pp.tile([128, t], F32, tag="ps")
                for k in range(nct):
                    nc.tensor.matmul(
                        ps,
                        w_tiles[k][:, m * 128 : (m + 1) * 128],
                        x_tiles[k],
                        start=(k == 0),
                        stop=(k == nct - 1),
                    )
                nc.scalar.activation(
                    dst, ps, AF.Identity, bias=b_sb[:, m : m + 1]
                )

            def emit_proj_group(yT_sb, b, m):
                ps = pp.tile([128, t], F32, tag="ps", name=f"pj{b}_{m}")
                for k in range(nct):
                    nc.tensor.matmul(
                        ps,
                        wp_k[k][:, m * 128 : (m + 1) * 128],
                        yT_sb[:, k, :],
                        start=(k == 0),
                        stop=(k == nct - 1),
                    )
                ot = opool.tile([128, t], BF16, tag="ot", name=f"ot{b}_{m}")
                nc.scalar.activation(
                    ot, ps, AF.Identity, bias=bpe_sb[:, m : m + 1]
                )
                eng = nc.sync if m % 2 == 0 else nc.scalar
                eng.dma_start(outT[b, m * 128 : (m + 1) * 128, :], ot)

            # prologue: Q/K projections of batch 0
            qT_cur, kT_cur = alloc_qkT(0)
            for m in range(nct):
                emit_lin_group(qT_cur[m], wq_k, x_cur, m, bq_sb)
            for m in range(nct):
                emit_lin_group(kT_cur[m], wk_k, x_cur, m, bk_sb)

            pending_proj = None  # (yT_sb, b, next_m)

            for b in range(bl):
                x_next = load_x(b + 1) if b + 1 < bl else None

                # ---- V projection (bf16) -> [t, c] layout + ones column ----
                v_sb = apool.tile([128, ktt, h, D + 1], BF16, tag="v", bufs=2)
                nc.vector.memset(v_sb[:, :, :, D : D + 1], 1.0)
                for tt in range(ktt):
                    for ch in range(nch):
                        cw = min(512, c - ch * 512)
                        ps = pp.tile([128, cw], F32, tag="ps")
                        for k in range(nct):
                            nc.tensor.matmul(
                                ps,
                                x_cur[k][:, tt * 128 : (tt + 1) * 128],
                                wv_k[k][:, ch * 512 : ch * 512 + cw],
                                start=(k == 0),
                                stop=(k == nct - 1),
                            )
                        nc.vector.tensor_copy(
                            v_sb[:, tt, ch * 8 : ch * 8 + cw // D, 0:D],
                            ps.rearrange("p (hh d) -> p hh d", d=D),
                        )

                if b + 1 < bl:
                    qT_next, kT_next = alloc_qkT(b + 1)
                yT_sb = apool.tile([128, nct, t], BF16, tag="yT", bufs=2)

                def stage_av(at, ct):
                    # AV matmuls for head pair ct (ones column -> denom row D)
                    avs = [
                        pp.tile(
                            [D + 1, t], F32, tag="av", bufs=3,
                            name=f"av{b}_{ct}_{s2}",
                        )
                        for s2 in range(2)
                    ]
                    for sub in range(2):
                        for i in range(ktt):
                            n = t - 128 * i
                            nc.tensor.matmul(
                                avs[sub][:, 128 * i : t],
                                v_sb[:, i, 2 * ct + sub, :],
                                at[:, i, sub, 0:n],
                                start=(i == 0),
                                stop=(i == ktt - 1),
                            )
                    return (avs, ct)

                def stage_norm(avs, ct):
                    # Runs 2 head pairs behind the AV fill.  approx-recip
                    # needs an SBUF source; evacuate the two denominator rows
                    # on ACT/DVE then one fused recip op.
                    den = npool.tile([1, 2, t], F32, tag="den", bufs=2)
                    nc.scalar.copy(den[:, 0, :], avs[0][D : D + 1, :])
                    nc.vector.tensor_copy(den[:, 1, :], avs[1][D : D + 1, :])
                    rec = npool.tile([1, 2, t], F32, tag="rec", bufs=2)
                    nc.vector.reciprocal_approx_fast(out=rec, in_=den)
                    # one broadcast to all 128 partitions (gpsimd ucode
                    # requires the output to start at partition 0)
                    rb = npool.tile([128, 2, t], F32, tag="rb", bufs=2)
                    nc.gpsimd.partition_broadcast(rb, rec, channels=128)
                    for sub in range(2):
                        po = sub * 64
                        nc.vector.tensor_tensor(
                            yT_sb[po : po + 64, ct, :],
                            avs[sub][0:D, :],
                            rb[po : po + 64, sub, :],
                            op=OP.mult,
                        )

                pend_av = None
                pend_norm = None
                for ct in range(nct):  # head pair (2*ct, 2*ct+1)
                    # S matmuls + exp for this pair; ONE merged mask multiply
                    at = attp.tile(
                        [128, ktt, 2, t], BF16, tag="at", name=f"at{b}_{ct}"
                    )
                    for i in range(ktt):
                        n = t - 128 * i
                        for sub in range(2):
                            po = sub * 64
                            ps_s = pp.tile([128, n], F32, tag="ss", bufs=3)
                            nc.tensor.matmul(
                                ps_s,
                                kT_cur[ct][po : po + 64, 128 * i : 128 * (i + 1)],
                                qT_cur[ct][po : po + 64, 128 * i : t],
                                start=True,
                                stop=True,
                            )
                            nc.scalar.activation(
                                at[:, i, sub, 0:n],
                                ps_s,
                                AF.Exp,
                                bias=pad_sb[:, b * ktt + i : b * ktt + i + 1],
                                scale=scale,
                            )
                    nc.vector.tensor_tensor(
                        at[:, :, :, 0:128], at[:, :, :, 0:128], cm_sb,
                        op=OP.mult,
                    )
                    # PE fill: out-proj of batch b-1, Q/K proj of batch b+1
                    n_fill = 1 if b + 1 < bl else 2
                    for _ in range(n_fill):
                        if (
                            pending_proj is not None
                            and ct >= 1
                            and pending_proj[2] < nct
                        ):
                            emit_proj_group(
                                pending_proj[0], pending_proj[1], pending_proj[2]
                            )
                            pending_proj = (
                                pending_proj[0],
                                pending_proj[1],
                                pending_proj[2] + 1,
                            )
                    if b + 1 < bl:
                        emit_lin_group(qT_next[ct], wq_k, x_next, ct, bq_sb)
                        emit_lin_group(kT_next[ct], wk_k, x_next, ct, bk_sb)
                    if pend_norm is not None:
                        stage_norm(*pend_norm)
                        pend_norm = None
                    if pend_av is not None:
                        pend_norm = stage_av(*pend_av)
                    pend_av = (at, ct)
                # drain the attention pipeline; leftover proj groups of the
                # previous batch go first so the PE has work while the last
                # head pair's exp/mask chain completes
                if pending_proj is not None:
                    for m in range(pending_proj[2], nct):
                        emit_proj_group(pending_proj[0], pending_proj[1], m)
                if pend_norm is not None:
                    stage_norm(*pend_norm)
                pend_norm = stage_av(*pend_av)
                stage_norm(*pend_norm)
                pending_proj = (yT_sb, b, 0)
                if b + 1 < bl:
                    x_cur = x_next
                    qT_cur, kT_cur = qT_next, kT_next

            for m in range(pending_proj[2], nct):
                emit_proj_group(pending_proj[0], pending_proj[1], m)

    nc.compile()
    return nc


def _prep_core_inputs(x_local, kpm_local, c=C, t=T, bl=BL):
    """Host-side packing of one core's inputs."""
    ktt = t // 128
    xT = np.ascontiguousarray(
        np.asarray(x_local, dtype=np.float32).transpose(2, 0, 1).reshape(c, bl * t)
    )
    pad = np.where(kpm_local, np.float32(NEG), np.float32(0.0)).astype(np.float32)
    # pad_t[p, b*ktt + i] = pad[b, i*128 + p]
    pad_t = np.ascontiguousarray(
        pad.reshape(bl, ktt, 128).transpose(2, 0, 1).reshape(128, bl * ktt)
    )
    return {"xTb": xT.astype(BF16_NP), "pad_t": pad_t}


def _prep_shared_inputs(Wq, bq, Wk, bk, Wv, bv, Wp, bp, c=C, t=T):
    nct = c // 128
    ktt = t // 128
    Wq = np.asarray(Wq, dtype=np.float32)
    Wk = np.asarray(Wk, dtype=np.float32)
    Wv = np.asarray(Wv, dtype=np.float32)
    Wp = np.asarray(Wp, dtype=np.float32)
    bq = np.asarray(bq, dtype=np.float32)
    bk = np.asarray(bk, dtype=np.float32)
    bv = np.asarray(bv, dtype=np.float32)
    bp = np.asarray(bp, dtype=np.float32)
    bp_eff = bp + Wp @ bv
    # causal 0/1 mask for a diagonal 128x128 block in [k, q] layout,
    # replicated over (key-tile, head-sub) for the merged mask multiply
    cm1 = (np.arange(128)[:, None] <= np.arange(128)[None, :]).astype(BF16_NP)
    cm = np.ascontiguousarray(
        np.broadcast_to(cm1[:, None, None, :], (128, ktt, 2, 128))
    )

    def btile(v):
        return np.ascontiguousarray(v.reshape(nct, 128).T)

    return {
        "wq_t": np.ascontiguousarray(Wq.T.astype(BF16_NP)),
        "wk_t": np.ascontiguousarray(Wk.T.astype(BF16_NP)),
        "wv_t": np.ascontiguousarray(Wv.T.astype(BF16_NP)),
        "wp_t": np.ascontiguousarray(Wp.T.astype(BF16_NP)),
        "bq_t": btile(bq),
        "bk_t": btile(bk),
        "bpe_t": btile(bp_eff),
        "cmask": cm,
    }


_NC_CACHE = {}


def _get_nc(key=(C, T, BL, H)):
    if key not in _NC_CACHE:
        _NC_CACHE[key] = build_nc(*key)
    return _NC_CACHE[key]


LAST_RESULT = None  # test harness reads exec_time_ns / trace path from here


def kernel(
    x, key_padding_mask, Wq, bq, Wk, bk, Wv, bv, Wp, bp,
    _trace=False, _trace_kwargs=None,
):
    global LAST_RESULT
    from concourse.bass_utils import run_bass_kernel_spmd

    x = np.asarray(x, dtype=np.float32)
    kpm = np.asarray(key_padding_mask).astype(bool)

    shared = _prep_shared_inputs(Wq, bq, Wk, bk, Wv, bv, Wp, bp)
    in_maps = []
    for cid in range(N_CORES):
        sl = slice(cid * BL, (cid + 1) * BL)
        m = _prep_core_inputs(x[sl], kpm[sl])
        m.update(shared)
        in_maps.append(m)

    nc = _get_nc()
    kw = {}
    if _trace:
        kw = dict(trace=True, trace_cores=[0], trace_kwargs=_trace_kwargs or {})
    res = run_bass_kernel_spmd(nc, in_maps, core_ids=list(range(N_CORES)), **kw)
    LAST_RESULT = res

    out = np.empty((B, T, C), dtype=np.float32)
    for cid in range(N_CORES):
        o = np.asarray(res.results[cid]["outT"], dtype=np.float32)  # [BL, C, T]
        out[cid * BL : (cid + 1) * BL] = o.transpose(0, 2, 1)
    return out


# revision 21
# speedup vs baseline: 1.0427x; 1.0427x over previous
"""Causal multi-head self-attention (B=32, T=512, C=1024, H=16) on 8 trn2 cores.

Data-parallel over batch (4 items/core), identical NEFF on all cores.
All activations stay in [channel, token] layout so every matmul has its
contraction dim on partitions with no transposes:

  qT/kT = Wq/Wk-tiles.T @ xT-tiles          (bf16, 512-wide free dim)
  S_T   = K_slice.T @ Q_slice  [k, q]       (bf16; causal => shrink N per kt;
                                             head pairs in PE row groups)
  att   = exp(scale*S_T + pad_bias[k])      (ACT; pad mask as per-part. bias)
  att  *= causal_binmask (diag blocks)      (one merged DVE multiply per pair)
  y/den = [V | 1].T @ att                   (bf16; ones column -> denom)
  rec   = 1/den                             (DVE reciprocal_approx_fast)
  rb    = bcast(rec)                        (GpSimd partition_broadcast)
  yT    = av_psum * rb                      (DVE, fused evac+normalize)
  outT  = Wp-tiles.T @ yT + bp_eff          (bf16; bv folded into bp_eff)

Schedule: the Q/K projections of batch b+1 and the output projection of
batch b-1 are interleaved into the attention head-pair loop of batch b,
one m-group of each per head pair.  This balances the phases: the
attention loop is PE-bound with ACT/DVE slack, instead of a PE-dense
QKV phase followed by an ACT/DVE-bound attention phase.  The AV fill
runs one head pair behind the S/exp stage and the softmax-normalize
chain two behind, so no engine queue head-of-line blocks on a
just-emitted matmul.  gpsimd runs ONLY partition_broadcast — mixing in
any default-library op makes it thrash ucode libraries (LOAD_LIB /
UNLOAD_LIB with full drains every iteration).
"""

import sys

sys.path.insert(0, "/opt/trn_rl_repo")

import ml_dtypes
import numpy as np

import concourse.bass as bass
import concourse.tile as tile
from concourse import bacc, mybir

B, T, C, H = 32, 512, 1024, 16
D = C // H  # 64
N_CORES = 8
BL = B // N_CORES  # batches per core
NEG = -1.0e9

F32 = mybir.dt.float32
BF16 = mybir.dt.bfloat16
BF16_NP = ml_dtypes.bfloat16
AF = mybir.ActivationFunctionType
OP = mybir.AluOpType


def build_nc(c=C, t=T, bl=BL, h=H):
    """Build the per-core Bass program. Same NEFF runs on every core."""
    nct = c // 128   # channel tiles
    ktt = t // 128   # key/token tiles per sequence
    nch = (c + 511) // 512  # 512-wide output chunks for V projection
    scale = 1.0 / float(np.sqrt(D))

    nc = bacc.Bacc(None, target_bir_lowering=False)

    xTb = nc.dram_tensor("xTb", [c, bl * t], BF16, kind="ExternalInput")
    wq_t = nc.dram_tensor("wq_t", [c, c], BF16, kind="ExternalInput")
    wk_t = nc.dram_tensor("wk_t", [c, c], BF16, kind="ExternalInput")
    wv_t = nc.dram_tensor("wv_t", [c, c], BF16, kind="ExternalInput")
    wp_t = nc.dram_tensor("wp_t", [c, c], BF16, kind="ExternalInput")
    bq_t = nc.dram_tensor("bq_t", [128, nct], F32, kind="ExternalInput")
    bk_t = nc.dram_tensor("bk_t", [128, nct], F32, kind="ExternalInput")
    bpe_t = nc.dram_tensor("bpe_t", [128, nct], F32, kind="ExternalInput")
    pad_t = nc.dram_tensor("pad_t", [128, bl * ktt], F32, kind="ExternalInput")
    cmask = nc.dram_tensor("cmask", [128, ktt, 2, 128], BF16, kind="ExternalInput")
    outT = nc.dram_tensor("outT", [bl, c, t], BF16, kind="ExternalOutput")

    with tile.TileContext(nc) as tc:
        with (
            tc.tile_pool(name="weights", bufs=1) as wpool,
            tc.tile_pool(name="consts", bufs=1) as cpool,
            tc.tile_pool(name="acts", bufs=1) as apool,
            tc.tile_pool(name="att", bufs=2) as attp,
            tc.tile_pool(name="norm", bufs=2) as npool,
            tc.tile_pool(name="oevac", bufs=2) as opool,
            tc.tile_pool(name="psum", bufs=2, space=bass.MemorySpace.PSUM) as pp,
        ):
            # ---- consts (tiny) + wq + wk on sync; x(b0) + wv/wp on scalar —
            #      the two HWDGE queues load in parallel
            bq_sb = cpool.tile([128, nct], F32, tag="bq")
            bk_sb = cpool.tile([128, nct], F32, tag="bk")
            bpe_sb = cpool.tile([128, nct], F32, tag="bpe")
            pad_sb = cpool.tile([128, bl * ktt], F32, tag="pad")
            cm_sb = cpool.tile([128, ktt, 2, 128], BF16, tag="cmask")
            nc.sync.dma_start(bq_sb, bq_t[:])
            nc.sync.dma_start(bk_sb, bk_t[:])
            nc.sync.dma_start(bpe_sb, bpe_t[:])
            nc.sync.dma_start(pad_sb, pad_t[:])
            nc.sync.dma_start(cm_sb, cmask[:])

            def load_w(dram, tag, eng):
                tiles = [
                    wpool.tile([128, c], BF16, tag=f"{tag}{k}", name=f"{tag}{k}")
                    for k in range(nct)
                ]
                w_r = dram[:].rearrange("(k p) m -> p k m", p=128)
                for k in range(nct):
                    eng.dma_start(tiles[k], w_r[:, k, :])
                return tiles

            def load_x(b):
                tiles = [
                    apool.tile([128, t], BF16, tag=f"x{k}", bufs=2, name=f"x{b}_{k}")
                    for k in range(nct)
                ]
                x_r = xTb[:, b * t : (b + 1) * t].rearrange(
                    "(k p) n -> p k n", p=128
                )
                for k in range(nct):
                    nc.sync.dma_start(tiles[k], x_r[:, k, :])
                return tiles

            def load_w_split(dram, tag):
                """Alternate k-tiles across both HWDGE queues."""
                tiles = [
                    wpool.tile([128, c], BF16, tag=f"{tag}{k}", name=f"{tag}{k}")
                    for k in range(nct)
                ]
                w_r = dram[:].rearrange("(k p) m -> p k m", p=128)
                for k in range(nct):
                    eng = nc.sync if k % 2 == 0 else nc.scalar
                    eng.dma_start(tiles[k], w_r[:, k, :])
                return tiles

            wq_k = load_w(wq_t, "wq", nc.sync)
            x_cur = load_x(0)
            wk_k = load_w(wk_t, "wk", nc.sync)
            wv_k = load_w(wv_t, "wv", nc.sync)
            wp_k = load_w(wp_t, "wp", nc.sync)

            def alloc_qkT(b):
                qTn = [
                    apool.tile([128, t], BF16, tag=f"qT{m}", bufs=2,
                               name=f"qT{b}_{m}")
                    for m in range(nct)
                ]
                kTn = [
                    apool.tile([128, t], BF16, tag=f"kT{m}", bufs=2,
                               name=f"kT{b}_{m}")
                    for m in range(nct)
                ]
                return qTn, kTn

            def emit_lin_group(dst, w_tiles, x_tiles, m, b_sb):
                """dst[128,t] = (W.T @ x)[m-block] + bias, evac on ACT."""
                ps = pp.tile([128, t], F32, tag="ps")
                for k in range(nct):
                    nc.tensor.matmul(
                        ps,
                        w_tiles[k][:, m * 128 : (m + 1) * 128],
                        x_tiles[k],
                        start=(k == 0),
                        stop=(k == nct - 1),
                    )
                nc.scalar.activation(
                    dst, ps, AF.Identity, bias=b_sb[:, m : m + 1]
                )

            def emit_proj_group(yT_sb, b, m):
                ps = pp.tile([128, t], F32, tag="ps", name=f"pj{b}_{m}")
                for k in range(nct):
                    nc.tensor.matmul(
                        ps,
                        wp_k[k][:, m * 128 : (m + 1) * 128],
                        yT_sb[:, k, :],
                        start=(k == 0),
                        stop=(k == nct - 1),
                    )
                ot = opool.tile([128, t], BF16, tag="ot", name=f"ot{b}_{m}")
                nc.scalar.activation(
                    ot, ps, AF.Identity, bias=bpe_sb[:, m : m + 1]
                )
                nc.sync.dma_start(outT[b, m * 128 : (m + 1) * 128, :], ot)

            # prologue: Q/K projections of batch 0
            qT_cur, kT_cur = alloc_qkT(0)
            for m in range(nct):
                emit_lin_group(qT_cur[m], wq_k, x_cur, m, bq_sb)
            for m in range(nct):
                emit_lin_group(kT_cur[m], wk_k, x_cur, m, bk_sb)

            pending_proj = None  # (yT_sb, b, next_m)

            for b in range(bl):
                x_next = load_x(b + 1) if b + 1 < bl else None

                # ---- V projection (bf16) -> [t, c] layout + ones column ----
                v_sb = apool.tile([128, ktt, h, D + 1], BF16, tag="v", bufs=2)
                nc.vector.memset(v_sb[:, :, :, D : D + 1], 1.0)
                for tt in range(ktt):
                    for ch in range(nch):
                        cw = min(512, c - ch * 512)
                        ps = pp.tile([128, cw], F32, tag="ps")
                        for k in range(nct):
                            nc.tensor.matmul(
                                ps,
                                x_cur[k][:, tt * 128 : (tt + 1) * 128],
                                wv_k[k][:, ch * 512 : ch * 512 + cw],
                                start=(k == 0),
                                stop=(k == nct - 1),
                            )
                        nc.vector.tensor_copy(
                            v_sb[:, tt, ch * 8 : ch * 8 + cw // D, 0:D],
                            ps.rearrange("p (hh d) -> p hh d", d=D),
                        )

                if b + 1 < bl:
                    qT_next, kT_next = alloc_qkT(b + 1)
                yT_sb = apool.tile([128, nct, t], BF16, tag="yT", bufs=2)

                def stage_av(at, ct):
                    # AV matmuls for head pair ct (ones column -> denom row D)
                    avs = [
                        pp.tile(
                            [D + 1, t], F32, tag="av", bufs=3,
                            name=f"av{b}_{ct}_{s2}",
                        )
                        for s2 in range(2)
                    ]
                    for sub in range(2):
                        for i in range(ktt):
                            n = t - 128 * i
                            nc.tensor.matmul(
                                avs[sub][:, 128 * i : t],
                                v_sb[:, i, 2 * ct + sub, :],
                                at[:, i, sub, 0:n],
                                start=(i == 0),
                                stop=(i == ktt - 1),
                            )
                    return (avs, ct)

                def stage_norm(avs, ct):
                    # Runs 2 head pairs behind the AV fill.  approx-recip
                    # needs an SBUF source; evacuate the two denominator rows
                    # on ACT/DVE then one fused recip op.
                    den = npool.tile([1, 2, t], F32, tag="den", bufs=2)
                    nc.scalar.copy(den[:, 0, :], avs[0][D : D + 1, :])
                    nc.vector.tensor_copy(den[:, 1, :], avs[1][D : D + 1, :])
                    rec = npool.tile([1, 2, t], F32, tag="rec", bufs=2)
                    nc.vector.reciprocal_approx_fast(out=rec, in_=den)
                    # one broadcast to all 128 partitions (gpsimd ucode
                    # requires the output to start at partition 0)
                    rb = npool.tile([128, 2, t], F32, tag="rb", bufs=2)
                    nc.gpsimd.partition_broadcast(rb, rec, channels=128)
                    for sub in range(2):
                        po = sub * 64
                        nc.vector.tensor_tensor(
                            yT_sb[po : po + 64, ct, :],
                            avs[sub][0:D, :],
                            rb[po : po + 64, sub, :],
                            op=OP.mult,
                        )

                pend_av = None
                pend_norm = None
                for ct in range(nct):  # head pair (2*ct, 2*ct+1)
                    # S matmuls + exp for this pair; ONE merged mask multiply
                    at = attp.tile(
                        [128, ktt, 2, t], BF16, tag="at", name=f"at{b}_{ct}"
                    )
                    for i in range(ktt):
                        n = t - 128 * i
                        for sub in range(2):
                            po = sub * 64
                            ps_s = pp.tile([128, n], F32, tag="ss", bufs=3)
                            nc.tensor.matmul(
                                ps_s,
                                kT_cur[ct][po : po + 64, 128 * i : 128 * (i + 1)],
                                qT_cur[ct][po : po + 64, 128 * i : t],
                                start=True,
                                stop=True,
                            )
                            nc.scalar.activation(
                                at[:, i, sub, 0:n],
                                ps_s,
                                AF.Exp,
                                bias=pad_sb[:, b * ktt + i : b * ktt + i + 1],
                                scale=scale,
                            )
                    nc.vector.tensor_tensor(
                        at[:, :, :, 0:128], at[:, :, :, 0:128], cm_sb,
                        op=OP.mult,
                    )
                    # PE fill: out-proj of batch b-1, Q/K proj of batch b+1
                    n_fill = 1 if b + 1 < bl else 2
                    for _ in range(n_fill):
                        if (
                            pending_proj is not None
                            and ct >= 1
                            and pending_proj[2] < nct
                        ):
                            emit_proj_group(
                                pending_proj[0], pending_proj[1], pending_proj[2]
                            )
                            pending_proj = (
                                pending_proj[0],
                                pending_proj[1],
                                pending_proj[2] + 1,
                            )
                    if b + 1 < bl:
                        emit_lin_group(qT_next[ct], wq_k, x_next, ct, bq_sb)
                        emit_lin_group(kT_next[ct], wk_k, x_next, ct, bk_sb)
                    if pend_norm is not None:
                        stage_norm(*pend_norm)
                        pend_norm = None
                    if pend_av is not None:
                        pend_norm = stage_av(*pend_av)
                    pend_av = (at, ct)
                # drain the attention pipeline; leftover proj groups of the
                # previous batch go first so the PE has work while the last
                # head pair's exp/mask chain completes
                if pending_proj is not None:
                    for m in range(pending_proj[2], nct):
                        emit_proj_group(pending_proj[0], pending_proj[1], m)
                if pend_norm is not None:
                    stage_norm(*pend_norm)
                pend_norm = stage_av(*pend_av)
                stage_norm(*pend_norm)
                pending_proj = (yT_sb, b, 0)
                if b + 1 < bl:
                    x_cur = x_next
                    qT_cur, kT_cur = qT_next, kT_next

            for m in range(pending_proj[2], nct):
                emit_proj_group(pending_proj[0], pending_proj[1], m)

    nc.compile()
    return nc


def _prep_core_inputs(x_local, kpm_local, c=C, t=T, bl=BL):
    """Host-side packing of one core's inputs."""
    ktt = t // 128
    xT = np.ascontiguousarray(
        np.asarray(x_local, dtype=np.float32).transpose(2, 0, 1).reshape(c, bl * t)
    )
    pad = np.where(kpm_local, np.float32(NEG), np.float32(0.0)).astype(np.float32)
    # pad_t[p, b*ktt + i] = pad[b, i*128 + p]
    pad_t = np.ascontiguousarray(
        pad.reshape(bl, ktt, 128).transpose(2, 0, 1).reshape(128, bl * ktt)
    )
    return {"xTb": xT.astype(BF16_NP), "pad_t": pad_t}


def _prep_shared_inputs(Wq, bq, Wk, bk, Wv, bv, Wp, bp, c=C, t=T):
    nct = c // 128
    ktt = t // 128
    Wq = np.asarray(Wq, dtype=np.float32)
    Wk = np.asarray(Wk, dtype=np.float32)
    Wv = np.asarray(Wv, dtype=np.float32)
    Wp = np.asarray(Wp, dtype=np.float32)
    bq = np.asarray(bq, dtype=np.float32)
    bk = np.asarray(bk, dtype=np.float32)
    bv = np.asarray(bv, dtype=np.float32)
    bp = np.asarray(bp, dtype=np.float32)
    bp_eff = bp + Wp @ bv
    # causal 0/1 mask for a diagonal 128x128 block in [k, q] layout,
    # replicated over (key-tile, head-sub) for the merged mask multiply
    cm1 = (np.arange(128)[:, None] <= np.arange(128)[None, :]).astype(BF16_NP)
    cm = np.ascontiguousarray(
        np.broadcast_to(cm1[:, None, None, :], (128, ktt, 2, 128))
    )

    def btile(v):
        return np.ascontiguousarray(v.reshape(nct, 128).T)

    return {
        "wq_t": np.ascontiguousarray(Wq.T.astype(BF16_NP)),
        "wk_t": np.ascontiguousarray(Wk.T.astype(BF16_NP)),
        "wv_t": np.ascontiguousarray(Wv.T.astype(BF16_NP)),
        "wp_t": np.ascontiguousarray(Wp.T.astype(BF16_NP)),
        "bq_t": btile(bq),
        "bk_t": btile(bk),
        "bpe_t": btile(bp_eff),
        "cmask": cm,
    }


_NC_CACHE = {}


def _get_nc(key=(C, T, BL, H)):
    if key not in _NC_CACHE:
        _NC_CACHE[key] = build_nc(*key)
    return _NC_CACHE[key]


LAST_RESULT = None  # test harness reads exec_time_ns / trace path from here


def kernel(
    x, key_padding_mask, Wq, bq, Wk, bk, Wv, bv, Wp, bp,
    _trace=False, _trace_kwargs=None,
):
    global LAST_RESULT
    from concourse.bass_utils import run_bass_kernel_spmd

    x = np.asarray(x, dtype=np.float32)
    kpm = np.asarray(key_padding_mask).astype(bool)

    shared = _prep_shared_inputs(Wq, bq, Wk, bk, Wv, bv, Wp, bp)
    in_maps = []
    for cid in range(N_CORES):
        sl = slice(cid * BL, (cid + 1) * BL)
        m = _prep_core_inputs(x[sl], kpm[sl])
        m.update(shared)
        in_maps.append(m)

    nc = _get_nc()
    kw = {}
    if _trace:
        kw = dict(trace=True, trace_cores=[0], trace_kwargs=_trace_kwargs or {})
    res = run_bass_kernel_spmd(nc, in_maps, core_ids=list(range(N_CORES)), **kw)
    LAST_RESULT = res

    out = np.empty((B, T, C), dtype=np.float32)
    for cid in range(N_CORES):
        o = np.asarray(res.results[cid]["outT"], dtype=np.float32)  # [BL, C, T]
        out[cid * BL : (cid + 1) * BL] = o.transpose(0, 2, 1)
    return out


# revision 22
# speedup vs baseline: 1.0498x; 1.0068x over previous
"""Causal multi-head self-attention (B=32, T=512, C=1024, H=16) on 8 trn2 cores.

Data-parallel over batch (4 items/core), identical NEFF on all cores.
All activations stay in [channel, token] layout so every matmul has its
contraction dim on partitions with no transposes:

  qT/kT = Wq/Wk-tiles.T @ xT-tiles          (bf16, 512-wide free dim)
  S_T   = K_slice.T @ Q_slice  [k, q]       (bf16; causal => shrink N per kt;
                                             head pairs in PE row groups)
  att   = exp(scale*S_T + pad_bias[k])      (ACT; pad mask as per-part. bias)
  att  *= causal_binmask (diag blocks)      (one merged DVE multiply per pair)
  y/den = [V | 1].T @ att                   (bf16; ones column -> denom)
  rec   = 1/den                             (DVE reciprocal_approx_fast)
  rb    = bcast(rec)                        (GpSimd partition_broadcast)
  yT    = av_psum * rb                      (DVE, fused evac+normalize)
  outT  = Wp-tiles.T @ yT + bp_eff          (bf16; bv folded into bp_eff)

Schedule: the Q/K projections of batch b+1 and the output projection of
batch b-1 are interleaved into the attention head-pair loop of batch b,
one m-group of each per head pair.  This balances the phases: the
attention loop is PE-bound with ACT/DVE slack, instead of a PE-dense
QKV phase followed by an ACT/DVE-bound attention phase.  The AV fill
runs one head pair behind the S/exp stage and the softmax-normalize
chain two behind, so no engine queue head-of-line blocks on a
just-emitted matmul.  gpsimd runs ONLY partition_broadcast — mixing in
any default-library op makes it thrash ucode libraries (LOAD_LIB /
UNLOAD_LIB with full drains every iteration).
"""

import sys

sys.path.insert(0, "/opt/trn_rl_repo")

import ml_dtypes
import numpy as np

import concourse.bass as bass
import concourse.tile as tile
from concourse import bacc, mybir

B, T, C, H = 32, 512, 1024, 16
D = C // H  # 64
N_CORES = 8
BL = B // N_CORES  # batches per core
NEG = -1.0e9

F32 = mybir.dt.float32
BF16 = mybir.dt.bfloat16
BF16_NP = ml_dtypes.bfloat16
AF = mybir.ActivationFunctionType
OP = mybir.AluOpType


def build_nc(c=C, t=T, bl=BL, h=H):
    """Build the per-core Bass program. Same NEFF runs on every core."""
    nct = c // 128   # channel tiles
    ktt = t // 128   # key/token tiles per sequence
    nch = (c + 511) // 512  # 512-wide output chunks for V projection
    scale = 1.0 / float(np.sqrt(D))

    nc = bacc.Bacc(None, target_bir_lowering=False)

    xTb = nc.dram_tensor("xTb", [c, bl * t], BF16, kind="ExternalInput")
    wq_t = nc.dram_tensor("wq_t", [c, c], BF16, kind="ExternalInput")
    wk_t = nc.dram_tensor("wk_t", [c, c], BF16, kind="ExternalInput")
    wv_t = nc.dram_tensor("wv_t", [c, c], BF16, kind="ExternalInput")
    wp_t = nc.dram_tensor("wp_t", [c, c], BF16, kind="ExternalInput")
    bq_t = nc.dram_tensor("bq_t", [128, nct], F32, kind="ExternalInput")
    bk_t = nc.dram_tensor("bk_t", [128, nct], F32, kind="ExternalInput")
    bpe_t = nc.dram_tensor("bpe_t", [128, nct], F32, kind="ExternalInput")
    pad_t = nc.dram_tensor("pad_t", [128, bl * ktt], F32, kind="ExternalInput")
    cmask = nc.dram_tensor("cmask", [128, ktt, 2, 128], BF16, kind="ExternalInput")
    outT = nc.dram_tensor("outT", [bl, c, t], BF16, kind="ExternalOutput")

    with tile.TileContext(nc) as tc:
        with (
            tc.tile_pool(name="weights", bufs=1) as wpool,
            tc.tile_pool(name="consts", bufs=1) as cpool,
            tc.tile_pool(name="acts", bufs=1) as apool,
            tc.tile_pool(name="att", bufs=2) as attp,
            tc.tile_pool(name="norm", bufs=2) as npool,
            tc.tile_pool(name="oevac", bufs=2) as opool,
            tc.tile_pool(name="psum", bufs=2, space=bass.MemorySpace.PSUM) as pp,
        ):
            # ---- consts (tiny) + wq + wk on sync; x(b0) + wv/wp on scalar —
            #      the two HWDGE queues load in parallel
            bq_sb = cpool.tile([128, nct], F32, tag="bq")
            bk_sb = cpool.tile([128, nct], F32, tag="bk")
            bpe_sb = cpool.tile([128, nct], F32, tag="bpe")
            pad_sb = cpool.tile([128, bl * ktt], F32, tag="pad")
            cm_sb = cpool.tile([128, ktt, 2, 128], BF16, tag="cmask")
            nc.sync.dma_start(bq_sb, bq_t[:])
            nc.sync.dma_start(bk_sb, bk_t[:])
            nc.sync.dma_start(bpe_sb, bpe_t[:])
            nc.sync.dma_start(pad_sb, pad_t[:])
            nc.sync.dma_start(cm_sb, cmask[:])

            def load_w(dram, tag, eng):
                tiles = [
                    wpool.tile([128, c], BF16, tag=f"{tag}{k}", name=f"{tag}{k}")
                    for k in range(nct)
                ]
                w_r = dram[:].rearrange("(k p) m -> p k m", p=128)
                for k in range(nct):
                    eng.dma_start(tiles[k], w_r[:, k, :])
                return tiles

            def load_x(b):
                tiles = [
                    apool.tile([128, t], BF16, tag=f"x{k}", bufs=2, name=f"x{b}_{k}")
                    for k in range(nct)
                ]
                x_r = xTb[:, b * t : (b + 1) * t].rearrange(
                    "(k p) n -> p k n", p=128
                )
                eng = nc.scalar if b == 0 else nc.sync
                for k in range(nct):
                    eng.dma_start(tiles[k], x_r[:, k, :])
                return tiles

            def load_w_split(dram, tag):
                """Alternate k-tiles across both HWDGE queues."""
                tiles = [
                    wpool.tile([128, c], BF16, tag=f"{tag}{k}", name=f"{tag}{k}")
                    for k in range(nct)
                ]
                w_r = dram[:].rearrange("(k p) m -> p k m", p=128)
                for k in range(nct):
                    eng = nc.sync if k % 2 == 0 else nc.scalar
                    eng.dma_start(tiles[k], w_r[:, k, :])
                return tiles

            wq_k = load_w(wq_t, "wq", nc.sync)
            x_cur = load_x(0)
            wk_k = load_w(wk_t, "wk", nc.sync)
            wv_k = load_w(wv_t, "wv", nc.sync)
            wp_k = load_w(wp_t, "wp", nc.sync)

            def alloc_qkT(b):
                qTn = [
                    apool.tile([128, t], BF16, tag=f"qT{m}", bufs=2,
                               name=f"qT{b}_{m}")
                    for m in range(nct)
                ]
                kTn = [
                    apool.tile([128, t], BF16, tag=f"kT{m}", bufs=2,
                               name=f"kT{b}_{m}")
                    for m in range(nct)
                ]
                return qTn, kTn

            def emit_lin_group(dst, w_tiles, x_tiles, m, b_sb):
                """dst[128,t] = (W.T @ x)[m-block] + bias, evac on ACT."""
                ps = pp.tile([128, t], F32, tag="ps")
                for k in range(nct):
                    nc.tensor.matmul(
                        ps,
                        w_tiles[k][:, m * 128 : (m + 1) * 128],
                        x_tiles[k],
                        start=(k == 0),
                        stop=(k == nct - 1),
                    )
                nc.scalar.activation(
                    dst, ps, AF.Identity, bias=b_sb[:, m : m + 1]
                )

            def emit_proj_group(yT_sb, b, m):
                ps = pp.tile([128, t], F32, tag="ps", name=f"pj{b}_{m}")
                for k in range(nct):
                    nc.tensor.matmul(
                        ps,
                        wp_k[k][:, m * 128 : (m + 1) * 128],
                        yT_sb[:, k, :],
                        start=(k == 0),
                        stop=(k == nct - 1),
                    )
                ot = opool.tile([128, t], BF16, tag="ot", name=f"ot{b}_{m}")
                nc.scalar.activation(
                    ot, ps, AF.Identity, bias=bpe_sb[:, m : m + 1]
                )
                nc.sync.dma_start(outT[b, m * 128 : (m + 1) * 128, :], ot)

            # prologue: Q/K projections of batch 0
            qT_cur, kT_cur = alloc_qkT(0)
            for m in range(nct):
                emit_lin_group(qT_cur[m], wq_k, x_cur, m, bq_sb)
            for m in range(nct):
                emit_lin_group(kT_cur[m], wk_k, x_cur, m, bk_sb)

            pending_proj = None  # (yT_sb, b, next_m)

            for b in range(bl):
                x_next = load_x(b + 1) if b + 1 < bl else None

                # ---- V projection (bf16) -> [t, c] layout + ones column ----
                v_sb = apool.tile([128, ktt, h, D + 1], BF16, tag="v", bufs=2)
                nc.vector.memset(v_sb[:, :, :, D : D + 1], 1.0)
                for tt in range(ktt):
                    for ch in range(nch):
                        cw = min(512, c - ch * 512)
                        ps = pp.tile([128, cw], F32, tag="ps")
                        for k in range(nct):
                            nc.tensor.matmul(
                                ps,
                                x_cur[k][:, tt * 128 : (tt + 1) * 128],
                                wv_k[k][:, ch * 512 : ch * 512 + cw],
                                start=(k == 0),
                                stop=(k == nct - 1),
                            )
                        nc.vector.tensor_copy(
                            v_sb[:, tt, ch * 8 : ch * 8 + cw // D, 0:D],
                            ps.rearrange("p (hh d) -> p hh d", d=D),
                        )

                if b + 1 < bl:
                    qT_next, kT_next = alloc_qkT(b + 1)
                yT_sb = apool.tile([128, nct, t], BF16, tag="yT", bufs=2)

                def stage_av(at, ct):
                    # AV matmuls for head pair ct (ones column -> denom row D)
                    avs = [
                        pp.tile(
                            [D + 1, t], F32, tag="av", bufs=3,
                            name=f"av{b}_{ct}_{s2}",
                        )
                        for s2 in range(2)
                    ]
                    for sub in range(2):
                        for i in range(ktt):
                            n = t - 128 * i
                            nc.tensor.matmul(
                                avs[sub][:, 128 * i : t],
                                v_sb[:, i, 2 * ct + sub, :],
                                at[:, i, sub, 0:n],
                                start=(i == 0),
                                stop=(i == ktt - 1),
                            )
                    return (avs, ct)

                def stage_norm(avs, ct):
                    # Runs 2 head pairs behind the AV fill.  approx-recip
                    # needs an SBUF source; evacuate the two denominator rows
                    # on ACT/DVE then one fused recip op.
                    den = npool.tile([1, 2, t], F32, tag="den", bufs=2)
                    nc.scalar.copy(den[:, 0, :], avs[0][D : D + 1, :])
                    nc.vector.tensor_copy(den[:, 1, :], avs[1][D : D + 1, :])
                    rec = npool.tile([1, 2, t], F32, tag="rec", bufs=2)
                    nc.vector.reciprocal_approx_fast(out=rec, in_=den)
                    # one broadcast to all 128 partitions (gpsimd ucode
                    # requires the output to start at partition 0)
                    rb = npool.tile([128, 2, t], F32, tag="rb", bufs=2)
                    nc.gpsimd.partition_broadcast(rb, rec, channels=128)
                    for sub in range(2):
                        po = sub * 64
                        nc.vector.tensor_tensor(
                            yT_sb[po : po + 64, ct, :],
                            avs[sub][0:D, :],
                            rb[po : po + 64, sub, :],
                            op=OP.mult,
                        )

                pend_av = None
                pend_norm = None
                for ct in range(nct):  # head pair (2*ct, 2*ct+1)
                    # S matmuls + exp for this pair; ONE merged mask multiply
                    at = attp.tile(
                        [128, ktt, 2, t], BF16, tag="at", name=f"at{b}_{ct}"
                    )
                    for i in range(ktt):
                        n = t - 128 * i
                        for sub in range(2):
                            po = sub * 64
                            ps_s = pp.tile([128, n], F32, tag="ss", bufs=3)
                            nc.tensor.matmul(
                                ps_s,
                                kT_cur[ct][po : po + 64, 128 * i : 128 * (i + 1)],
                                qT_cur[ct][po : po + 64, 128 * i : t],
                                start=True,
                                stop=True,
                            )
                            nc.scalar.activation(
                                at[:, i, sub, 0:n],
                                ps_s,
                                AF.Exp,
                                bias=pad_sb[:, b * ktt + i : b * ktt + i + 1],
                                scale=scale,
                            )
                    nc.vector.tensor_tensor(
                        at[:, :, :, 0:128], at[:, :, :, 0:128], cm_sb,
                        op=OP.mult,
                    )
                    # PE fill: out-proj of batch b-1, Q/K proj of batch b+1
                    for _ in range(1):
                        if (
                            pending_proj is not None
                            and ct >= 1
                            and pending_proj[2] < nct
                        ):
                            emit_proj_group(
                                pending_proj[0], pending_proj[1], pending_proj[2]
                            )
                            pending_proj = (
                                pending_proj[0],
                                pending_proj[1],
                                pending_proj[2] + 1,
                            )
                    if b + 1 < bl:
                        emit_lin_group(qT_next[ct], wq_k, x_next, ct, bq_sb)
                        emit_lin_group(kT_next[ct], wk_k, x_next, ct, bk_sb)
                    if pend_norm is not None:
                        stage_norm(*pend_norm)
                        pend_norm = None
                    if pend_av is not None:
                        pend_norm = stage_av(*pend_av)
                    pend_av = (at, ct)
                # drain the attention pipeline; leftover proj groups of the
                # previous batch go first so the PE has work while the last
                # head pair's exp/mask chain completes
                if pending_proj is not None:
                    for m in range(pending_proj[2], nct):
                        emit_proj_group(pending_proj[0], pending_proj[1], m)
                if pend_norm is not None:
                    stage_norm(*pend_norm)
                pend_norm = stage_av(*pend_av)
                stage_norm(*pend_norm)
                pending_proj = (yT_sb, b, 0)
                if b + 1 < bl:
                    x_cur = x_next
                    qT_cur, kT_cur = qT_next, kT_next

            for m in range(pending_proj[2], nct):
                emit_proj_group(pending_proj[0], pending_proj[1], m)

    nc.compile()
    return nc


def _prep_core_inputs(x_local, kpm_local, c=C, t=T, bl=BL):
    """Host-side packing of one core's inputs."""
    ktt = t // 128
    xT = np.ascontiguousarray(
        np.asarray(x_local, dtype=np.float32).transpose(2, 0, 1).reshape(c, bl * t)
    )
    pad = np.where(kpm_local, np.float32(NEG), np.float32(0.0)).astype(np.float32)
    # pad_t[p, b*ktt + i] = pad[b, i*128 + p]
    pad_t = np.ascontiguousarray(
        pad.reshape(bl, ktt, 128).transpose(2, 0, 1).reshape(128, bl * ktt)
    )
    return {"xTb": xT.astype(BF16_NP), "pad_t": pad_t}


def _prep_shared_inputs(Wq, bq, Wk, bk, Wv, bv, Wp, bp, c=C, t=T):
    nct = c // 128
    ktt = t // 128
    Wq = np.asarray(Wq, dtype=np.float32)
    Wk = np.asarray(Wk, dtype=np.float32)
    Wv = np.asarray(Wv, dtype=np.float32)
    Wp = np.asarray(Wp, dtype=np.float32)
    bq = np.asarray(bq, dtype=np.float32)
    bk = np.asarray(bk, dtype=np.float32)
    bv = np.asarray(bv, dtype=np.float32)
    bp = np.asarray(bp, dtype=np.float32)
    bp_eff = bp + Wp @ bv
    # causal 0/1 mask for a diagonal 128x128 block in [k, q] layout,
    # replicated over (key-tile, head-sub) for the merged mask multiply
    cm1 = (np.arange(128)[:, None] <= np.arange(128)[None, :]).astype(BF16_NP)
    cm = np.ascontiguousarray(
        np.broadcast_to(cm1[:, None, None, :], (128, ktt, 2, 128))
    )

    def btile(v):
        return np.ascontiguousarray(v.reshape(nct, 128).T)

    return {
        "wq_t": np.ascontiguousarray(Wq.T.astype(BF16_NP)),
        "wk_t": np.ascontiguousarray(Wk.T.astype(BF16_NP)),
        "wv_t": np.ascontiguousarray(Wv.T.astype(BF16_NP)),
        "wp_t": np.ascontiguousarray(Wp.T.astype(BF16_NP)),
        "bq_t": btile(bq),
        "bk_t": btile(bk),
        "bpe_t": btile(bp_eff),
        "cmask": cm,
    }


_NC_CACHE = {}


def _get_nc(key=(C, T, BL, H)):
    if key not in _NC_CACHE:
        _NC_CACHE[key] = build_nc(*key)
    return _NC_CACHE[key]


LAST_RESULT = None  # test harness reads exec_time_ns / trace path from here


def kernel(
    x, key_padding_mask, Wq, bq, Wk, bk, Wv, bv, Wp, bp,
    _trace=False, _trace_kwargs=None,
):
    global LAST_RESULT
    from concourse.bass_utils import run_bass_kernel_spmd

    x = np.asarray(x, dtype=np.float32)
    kpm = np.asarray(key_padding_mask).astype(bool)

    shared = _prep_shared_inputs(Wq, bq, Wk, bk, Wv, bv, Wp, bp)
    in_maps = []
    for cid in range(N_CORES):
        sl = slice(cid * BL, (cid + 1) * BL)
        m = _prep_core_inputs(x[sl], kpm[sl])
        m.update(shared)
        in_maps.append(m)

    nc = _get_nc()
    kw = {}
    if _trace:
        kw = dict(trace=True, trace_cores=[0], trace_kwargs=_trace_kwargs or {})
    res = run_bass_kernel_spmd(nc, in_maps, core_ids=list(range(N_CORES)), **kw)
    LAST_RESULT = res

    out = np.empty((B, T, C), dtype=np.float32)
    for cid in range(N_CORES):
        o = np.asarray(res.results[cid]["outT"], dtype=np.float32)  # [BL, C, T]
        out[cid * BL : (cid + 1) * BL] = o.transpose(0, 2, 1)
    return out
